# revision 10
# baseline (speedup 1.0000x reference)
"""DIEN forward on 8 Trainium2 NeuronCores (Bass/Tile).

Data-parallel with ragged packing (v2: device-side embedding gather).
 - Host sorts batch rows by descending hist_valid_lens, deals them to the 8
   cores round-robin over the sorted order, and builds a shared per-timestep
   active-column schedule N_t (max over cores, padded to x4).  All per-step
   shapes are compile-time constants.
 - Upload per core is only ~1.2MB: an fp16 shard of the embedding table
   (AllGathered on device into DRAM), a packed f32 weight-blob shard
   (AllGathered likewise), int32 gather indices, dense features and lens.
 - On device: indirect-DMA gathers pull embedding rows (row-major, 128 per
   instruction), PE transposes flip them feature-major into the packed x
   grid / query / sparse tiles; attention masks are built from lens + iota.
 - Scans/attention/DNN head identical to v1: feature-major packed GRU and
   AUGRU scans over ragged columns, attention scattered into batch-major
   PSUM, BatchNorm stats via AllReduce.

kernel(**inputs) takes FULL unsharded inputs, returns [B, 1] float32.
"""

import hashlib
import numpy as np

import jax
# Persistent XLA compilation cache: run_bass_kernel_spmd builds a fresh jit
# closure per call, so the in-memory executable cache always misses and the
# BIR->NEFF compile would otherwise rerun on every invocation.
jax.config.update("jax_compilation_cache_dir", "/tmp/jax_bass_cache")
jax.config.update("jax_persistent_cache_min_entry_size_bytes", -1)
jax.config.update("jax_persistent_cache_min_compile_time_secs", 0)

import concourse.bass as bass
import concourse.bacc as bacc
import concourse.tile as tile
from concourse import mybir
from concourse.bass_utils import run_bass_kernel_spmd
from concourse.masks import make_identity

F32 = mybir.dt.float32
F16 = mybir.dt.float16
I32 = mybir.dt.int32
AF = mybir.ActivationFunctionType
OP = mybir.AluOpType
AX = mybir.AxisListType

B, T, E, NF, SL, DL, VOCAB = 2048, 200, 32, 2, 8, 16, 100000
D = E * NF          # 64
NCORES = 8
BC = B // NCORES    # 256
ESH = (VOCAB // NCORES) * E   # fp16 emb-table shard elements per core


# --------------------------------------------------------------------------
def _make_schedule(lens):
    order = np.argsort(-lens, kind="stable")
    core_lens = lens[order].reshape(-1, NCORES)       # [BC, 8]
    tmax = int(lens.max())
    nts = []
    for t in range(tmax):
        n = int((core_lens > t).sum(axis=0).max())
        n = min(BC, ((n + 3) // 4) * 4)
        nts.append(max(n, 4))
    nts = np.array(nts, np.int32)
    offs = np.zeros(tmax + 1, np.int64)
    offs[1:] = np.cumsum(nts)
    m_total = int(offs[tmax])
    n0 = int(nts[0])
    xcols = np.zeros(tmax, np.int64)
    xcols[1:] = n0 + offs[:tmax - 1]                  # x(t) at h(t-1) cols
    hcols = n0 + offs[:tmax]
    xspan = int(n0 + offs[tmax - 1]) if tmax > 1 else int(nts[0])
    xspan = max(xspan, int(xcols[tmax - 1] + nts[tmax - 1]))
    buf_cols = max(n0 + m_total, ((xspan + 127) // 128) * 128 + 128)
    buf_cols = ((buf_cols + 127) // 128) * 128
    return dict(order=order, tmax=tmax, nts=nts, offs=offs, m_total=m_total,
                n0=n0, xcols=xcols, hcols=hcols, xspan=xspan,
                buf_cols=buf_cols)


def _att_chunks(sch):
    chunks, cur, w = [], [], 0
    for t in range(sch["tmax"]):
        n = int(sch["nts"][t])
        if w + n > 512 and cur:
            chunks.append(cur)
            cur, w = [], 0
        cur.append(t)
        w += n
    if cur:
        chunks.append(cur)
    return chunks


# ------------------------- weight blob layout ------------------------------
WSEGS = [  # (name, partitions, cols)
    ("wrzg", 128, 128), ("wng", 128, 128), ("wrza", 128, 128),
    ("wna", 128, 128), ("gv", 128, 6), ("watt", 128, 3 * D),
    ("w2a", D, 16), ("w3a", 16, 1), ("ab", 64, 2), ("bng", 128, 6),
    ("w1t0", 128, 256), ("w1t1", 128, 256), ("w1t2", 128, 256),
    ("w2t0", 128, 128), ("w2t1", 128, 128),
    ("owt", 128, 1), ("dbt", 128, 3), ("obt", 1, 1),
]
WOFF = {}
_off = 0
for _n, _p, _c in WSEGS:
    WOFF[_n] = _off
    _off += _p * _c
WTOT = ((_off + NCORES - 1) // NCORES) * NCORES
WSH = WTOT // NCORES


# --------------------------------------------------------------------------
class _CachedBacc(bacc.Bacc):
    _json_cache = None

    def to_json_bytes(self):
        if self._json_cache is not None:
            return self._json_cache
        return super().to_json_bytes()


def _build(sch):
    nc = _CachedBacc("TRN2", target_bir_lowering=False, debug=False,
                     num_devices=NCORES)
    tmax, nts = sch["tmax"], sch["nts"]
    hcols, xcols = sch["hcols"], sch["xcols"]
    n0, buf_cols = sch["n0"], sch["buf_cols"]
    NXCH = buf_cols // 128          # x gather chunks (128 cols each)
    QOFF = 2 * NXCH                 # idx col offsets: q then sparse
    SOFF = QOFF + 4
    NIDX = SOFF + 16
    chunks = _att_chunks(sch)

    eshard = nc.dram_tensor("eshard", [1, ESH], F16, kind="ExternalInput")
    wshard = nc.dram_tensor("wshard", [1, 2 * WSH], F16, kind="ExternalInput")
    xidx = nc.dram_tensor("xidx", [128, NIDX], I32, kind="ExternalInput")
    densT = nc.dram_tensor("densT", [DL, BC], F32, kind="ExternalInput")
    lensv = nc.dram_tensor("lensv", [128, 2], F32, kind="ExternalInput")
    out = nc.dram_tensor("out", [1, BC], F32, kind="ExternalOutput")

    with tile.TileContext(nc) as tc:
        with tc.tile_pool(name="big", bufs=1) as big, \
             tc.tile_pool(name="w", bufs=1) as w, \
             tc.tile_pool(name="s", bufs=2) as sp, \
             tc.tile_pool(name="s2", bufs=2) as sp2, \
             tc.tile_pool(name="dram", bufs=1, space="DRAM") as dramp:

            BUF = big.tile([128, buf_cols], F32)
            RH2 = big.tile([128, BC], F32)
            qT = big.tile([128, BC], F32)
            spT = [big.tile([128, BC], F32, tag=f"spT{i}", name=f"spT{i}")
                   for i in range(2)]
            attB = [big.tile([128, 256], F32, tag=f"attB{i}", name=f"attB{i}")
                    for i in range(2)]
            attT = [big.tile([128, 256], F32, tag=f"attT{i}", name=f"attT{i}")
                    for i in range(2)]

            # ------------- collectives: gather table + weights ------------
            # (collectives cannot read IO tensors; stage via Internal DRAM)
            eint = dramp.tile([1, ESH], F16)
            nc.sync.dma_start(out=eint[:], in_=eshard[:])
            egt = dramp.tile([1, NCORES * ESH], F16)
            nc.gpsimd.collective_compute(
                "AllGather", OP.bypass,
                replica_groups=[list(range(NCORES))],
                ins=[eint[:].opt()], outs=[egt.opt()])
            etab = egt[0:1, :].rearrange("o (r e) -> (o r) e", e=E)

            # weight blob rides as raw bits in an f16 AllGather (an f32
            # AllGather next to the f16 one + indirect gathers wedges NRT)
            wint = dramp.tile([1, 2 * WSH], F16)
            nc.sync.dma_start(out=wint[:], in_=wshard[:])
            wgt = dramp.tile([1, NCORES * 2 * WSH], F16)
            nc.gpsimd.collective_compute(
                "AllGather", OP.bypass,
                replica_groups=[list(range(NCORES))],
                ins=[wint[:].opt()], outs=[wgt.opt()])
            wgt32 = wgt[0:1, :].bitcast(F32)

            def wload(dst, name, p, c):
                nc.sync.dma_start(
                    out=dst[0:p, 0:c],
                    in_=wgt32[0:1, WOFF[name]:WOFF[name] + p * c].rearrange(
                        "o (p c) -> (o p) c", p=p))

            ident = w.tile([128, 128], F32)
            make_identity(nc, ident[:])
            ident16 = w.tile([128, 128], F16)
            make_identity(nc, ident16[:])
            ones1 = w.tile([1, 64], F32)
            nc.vector.memset(ones1[:], 1.0)

            wrzg = w.tile([128, 128], F32)
            wng = w.tile([128, 128], F32)
            wrza = w.tile([128, 128], F32)
            wna = w.tile([128, 128], F32)
            gv = w.tile([128, 6], F32)
            watt = w.tile([128, 3 * D], F32)
            w2a = w.tile([D, 16], F32)
            w3a = w.tile([16, 1], F32)
            ab = w.tile([64, 2], F32)
            for nm, dst in (("wrzg", wrzg), ("wng", wng), ("wrza", wrza),
                            ("wna", wna), ("gv", gv), ("watt", watt),
                            ("w2a", w2a), ("w3a", w3a), ("ab", ab)):
                p, c = dict((s[0], (s[1], s[2])) for s in WSEGS)[nm]
                wload(dst, nm, p, c)

            # ------------- indices, lens, masks ---------------------------
            XI = w.tile([128, NIDX], I32)
            nc.sync.dma_start(out=XI[:], in_=xidx[:])
            LV = w.tile([128, 2], F32)
            nc.sync.dma_start(out=LV[:], in_=lensv[:])
            iotaF = w.tile([128, T], F32)
            nc.gpsimd.iota(iotaF[:], pattern=[[1, T]], base=0,
                           channel_multiplier=0,
                           allow_small_or_imprecise_dtypes=True)
            mskT = [w.tile([128, T], F32, tag=f"mskT{i}", name=f"mskT{i}")
                    for i in range(2)]
            for i in range(2):
                nc.vector.tensor_scalar(out=mskT[i][:], in0=iotaF[:],
                                        scalar1=LV[:, i:i + 1], scalar2=None,
                                        op0=OP.is_lt)
                nc.vector.tensor_scalar(out=mskT[i][:], in0=mskT[i][:],
                                        scalar1=1.0, scalar2=1e9,
                                        op0=OP.subtract, op1=OP.mult)

            nc.vector.memset(BUF[64:128, 0:n0], 0.0)

            # ------------- device-side embedding gathers ------------------
            def gather_pair(gp, gpsum, c0, c1):
                G = gp.tile([128, 64], F16, tag="G")
                nc.gpsimd.indirect_dma_start(
                    out=G[:, 0:E], out_offset=None, in_=etab,
                    in_offset=bass.IndirectOffsetOnAxis(
                        ap=XI[:, c0:c0 + 1], axis=0))
                nc.gpsimd.indirect_dma_start(
                    out=G[:, E:2 * E], out_offset=None, in_=etab,
                    in_offset=bass.IndirectOffsetOnAxis(
                        ap=XI[:, c1:c1 + 1], axis=0))
                pt = gpsum.tile([64, 128], F16, tag="pt")
                nc.tensor.transpose(out=pt[:], in_=G[:], identity=ident16[:])
                return pt

            with tc.tile_pool(name="g", bufs=4) as gp, \
                 tc.tile_pool(name="gps", bufs=4, space="PSUM") as gpsum:
                for c in range(NXCH):
                    pt = gather_pair(gp, gpsum, 2 * c, 2 * c + 1)
                    nc.vector.tensor_copy(
                        out=BUF[0:64, c * 128:(c + 1) * 128], in_=pt[:])
                for a in range(2):
                    pt = gather_pair(gp, gpsum, QOFF + 2 * a, QOFF + 2 * a + 1)
                    nc.vector.tensor_copy(
                        out=qT[64:128, a * 128:(a + 1) * 128], in_=pt[:])
                for a in range(2):
                    for j in range(4):
                        c0 = SOFF + a * 8 + 2 * j
                        pt = gather_pair(gp, gpsum, c0, c0 + 1)
                        nc.vector.tensor_copy(
                            out=spT[j // 2][(j % 2) * 64:(j % 2) * 64 + 64,
                                            a * 128:(a + 1) * 128],
                            in_=pt[:])

            # ---------------- scan step ---------------------------------
            def scan_step(pool, t, rhs_buf, rhs_col, wrz, wn, vo, out_buf,
                          out_col, att_rhs=None):
                n = int(nts[t])
                pA = pool.tile([128, 256], F32, tag="pA")
                pB = pool.tile([128, 256], F32, tag="pB")
                rhs = rhs_buf[:, rhs_col:rhs_col + n]
                nc.tensor.matmul(out=pA[:, 0:n], lhsT=wrz[:], rhs=rhs,
                                 start=True, stop=True)
                nc.tensor.matmul(out=pB[:, 0:n], lhsT=wn[:], rhs=rhs,
                                 start=True, stop=True)
                srz = sp.tile([128, 256], F32, tag="srz")
                nc.scalar.activation(out=srz[:, 0:n], in_=pA[:, 0:n],
                                     func=AF.Sigmoid,
                                     bias=gv[:, vo:vo + 1], scale=1.0)
                t1 = sp.tile([128, 256], F32, tag="t1")
                nc.vector.scalar_tensor_tensor(
                    out=t1[64:128, 0:n], in0=pB[64:128, 0:n],
                    scalar=gv[64:128, vo + 1:vo + 2],
                    in1=srz[64:128, 0:n], op0=OP.add, op1=OP.mult)
                t2 = sp.tile([128, 256], F32, tag="t2")
                nc.vector.tensor_tensor(out=t2[64:128, 0:n],
                                        in0=t1[64:128, 0:n],
                                        in1=pB[0:64, 0:n], op=OP.add)
                nt = sp.tile([128, 256], F32, tag="nt")
                nc.scalar.activation(out=nt[64:128, 0:n], in_=t2[64:128, 0:n],
                                     func=AF.Tanh,
                                     bias=gv[64:128, vo + 2:vo + 3], scale=1.0)
                pD = pool.tile([64, 256], F32, tag="pD")
                h_prev = rhs_buf[64:128, rhs_col:rhs_col + n]
                et = sp2.tile([128, 256], F32, tag="et")
                if att_rhs is None:
                    # GRU: h' = n + z*(h - n)
                    nc.vector.tensor_tensor(out=pD[0:64, 0:n], in0=h_prev,
                                            in1=nt[64:128, 0:n],
                                            op=OP.subtract)
                    nc.vector.tensor_tensor(out=et[64:128, 0:n],
                                            in0=pD[0:64, 0:n],
                                            in1=srz[0:64, 0:n], op=OP.mult)
                    nc.vector.tensor_tensor(
                        out=out_buf[64:128, out_col:out_col + n],
                        in0=et[64:128, 0:n], in1=nt[64:128, 0:n], op=OP.add)
                else:
                    # AUGRU: h' = h + att*z*(n - h)
                    nc.vector.tensor_tensor(out=pD[0:64, 0:n],
                                            in0=nt[64:128, 0:n],
                                            in1=h_prev, op=OP.subtract)
                    pAtt = pool.tile([64, 256], F32, tag="pAtt")
                    nc.tensor.matmul(out=pAtt[:, 0:n], lhsT=ones1[:],
                                     rhs=att_rhs, start=True, stop=True)
                    zt = sp2.tile([128, 256], F32, tag="zt")
                    nc.vector.tensor_tensor(out=zt[0:64, 0:n],
                                            in0=pAtt[0:64, 0:n],
                                            in1=srz[0:64, 0:n], op=OP.mult)
                    nc.vector.tensor_tensor(out=et[64:128, 0:n],
                                            in0=pD[0:64, 0:n],
                                            in1=zt[0:64, 0:n], op=OP.mult)
                    nc.vector.tensor_tensor(
                        out=out_buf[64:128, out_col:out_col + n],
                        in0=et[64:128, 0:n],
                        in1=rhs_buf[64:128, rhs_col:rhs_col + n], op=OP.add)

            # ---------------- GRU scan ----------------------------------
            with tc.tile_pool(name="sps", bufs=2, space="PSUM") as sps:
                for t in range(tmax):
                    scan_step(sps, t, BUF, int(xcols[t]), wrzg, wng, 0,
                              BUF, int(hcols[t]))

            # ---------------- attention ---------------------------------
            with tc.tile_pool(name="apsB", bufs=1, space="PSUM") as apsB, \
                 tc.tile_pool(name="aps", bufs=2, space="PSUM") as aps:
                psB = [apsB.tile([128, T], F32, tag=f"psB{i}", name=f"psB{i}")
                       for i in range(2)]
                nc.vector.memset(psB[0][:], 0.0)
                nc.vector.memset(psB[1][:], 0.0)

                for ch in chunks:
                    wch = int(sum(int(nts[t]) for t in ch))
                    qk = sp.tile([128, 512], F32, tag="qk")
                    col = 0
                    for t in ch:
                        n = int(nts[t])
                        hc = int(hcols[t])
                        nc.vector.tensor_tensor(
                            out=qk[64:128, col:col + n],
                            in0=BUF[64:128, hc:hc + n],
                            in1=qT[64:128, 0:n], op=OP.mult)
                        col += n
                    pL1 = aps.tile([64, 512], F32, tag="pL1")
                    col = 0
                    for t in ch:
                        n = int(nts[t])
                        hc = int(hcols[t])
                        nc.tensor.matmul(out=pL1[:, col:col + n],
                                         lhsT=watt[64:128, 0:64],
                                         rhs=BUF[64:128, hc:hc + n],
                                         start=True, stop=False)
                        nc.tensor.matmul(out=pL1[:, col:col + n],
                                         lhsT=watt[64:128, 64:128],
                                         rhs=qk[64:128, col:col + n],
                                         start=False, stop=False)
                        nc.tensor.matmul(out=pL1[:, col:col + n],
                                         lhsT=watt[64:128, 128:192],
                                         rhs=qT[64:128, 0:n],
                                         start=False, stop=True)
                        col += n
                    h1 = sp.tile([64, 512], F32, tag="h1")
                    nc.scalar.activation(out=h1[:, 0:wch], in_=pL1[:, 0:wch],
                                         func=AF.Relu, bias=ab[:, 0:1],
                                         scale=1.0)
                    pL2 = aps.tile([16, 512], F32, tag="pL2")
                    nc.tensor.matmul(out=pL2[:, 0:wch], lhsT=w2a[:],
                                     rhs=h1[:, 0:wch], start=True, stop=True)
                    h2 = sp.tile([16, 512], F32, tag="h2")
                    nc.scalar.activation(out=h2[:, 0:wch], in_=pL2[:, 0:wch],
                                         func=AF.Relu, bias=ab[0:16, 1:2],
                                         scale=1.0)
                    col = 0
                    for t in ch:
                        n = int(nts[t])
                        for piece in range(2):
                            lo = piece * 128
                            if lo >= n:
                                break
                            pw = min(128, n - lo)
                            nc.tensor.matmul(
                                out=psB[piece][0:pw, t:t + 1],
                                lhsT=h2[:, col + lo:col + lo + pw],
                                rhs=w3a[:], start=True, stop=True)
                        col += n

                # softmax (batch-major)
                for i in range(2):
                    sc_t = sp.tile([128, T], F32, tag="sct")
                    nc.vector.tensor_tensor(out=sc_t[:], in0=psB[i][:],
                                            in1=mskT[i][:], op=OP.add)
                    mx = sp.tile([128, 1], F32, tag="mx")
                    nc.vector.tensor_reduce(out=mx[:], in_=sc_t[:],
                                            axis=AX.X, op=OP.max)
                    nmx = sp.tile([128, 1], F32, tag="nmx")
                    nc.vector.tensor_scalar_mul(nmx[:], mx[:], -1.0)
                    ex = sp.tile([128, 256], F32, tag="ex")
                    nc.vector.memset(ex[:], 0.0)
                    nc.scalar.activation(out=ex[:, 0:T], in_=sc_t[:],
                                         func=AF.Exp, bias=nmx[:], scale=1.0)
                    sm = sp.tile([128, 1], F32, tag="sm")
                    nc.vector.tensor_reduce(out=sm[:], in_=ex[:, 0:T],
                                            axis=AX.X, op=OP.add)
                    rs = sp.tile([128, 1], F32, tag="rs")
                    nc.vector.reciprocal(out=rs[:], in_=sm[:])
                    nc.vector.memset(attB[i][:], 0.0)
                    nc.vector.tensor_scalar(
                        out=attB[i][:, 0:T], in0=ex[:, 0:T], scalar1=rs[:],
                        scalar2=None, op0=OP.mult)

                # transpose attB -> attT (rows = t, cols = r)
                for th in range(2):
                    tw = 128 if th == 0 else T - 128
                    for rh in range(2):
                        pat = aps.tile([128, 128], F32, tag="pAT")
                        nc.tensor.transpose(
                            out=pat[0:tw, :],
                            in_=attB[rh][:, th * 128:th * 128 + tw],
                            identity=ident[:])
                        nc.vector.tensor_copy(
                            out=attT[th][0:tw, rh * 128:(rh + 1) * 128],
                            in_=pat[0:tw, :])

            # ---------------- AUGRU scan --------------------------------
            nc.vector.memset(RH2[:], 0.0)
            with tc.tile_pool(name="aups", bufs=2, space="PSUM") as aups, \
                 tc.tile_pool(name="strp", bufs=2) as strp:
                nstrip = (tmax + 7) // 8
                for s in range(nstrip):
                    t0 = s * 8
                    t1s = min(t0 + 8, tmax)
                    rows = t1s - t0
                    strip = strp.tile([1, 8 * 256], F32, tag="strip")
                    th = t0 // 128
                    r0 = t0 - th * 128
                    nc.sync.dma_start(
                        out=strip[0:1, 0:rows * 256].rearrange(
                            "o (t r) -> o t r", t=rows),
                        in_=attT[th][r0:r0 + rows, :])
                    for t in range(t0, t1s):
                        n = int(nts[t])
                        hc = int(hcols[t])
                        nc.gpsimd.tensor_copy(out=RH2[0:64, 0:n],
                                              in_=BUF[64:128, hc:hc + n])
                        arhs = strip[0:1, (t - t0) * 256:(t - t0) * 256 + n]
                        scan_step(aups, t, RH2, 0, wrza, wna, 3, RH2, 0,
                                  att_rhs=arhs)

            # ---------------- DNN head ----------------------------------
            with tc.tile_pool(name="mps", bufs=2, space="PSUM") as mps:
                densTt = big.tile([128, BC], F32, tag="densTt")
                nc.vector.memset(densTt[:], 0.0)
                nc.sync.dma_start(out=densTt[0:DL, :], in_=densT[:])
                nc.vector.tensor_copy(out=densTt[64:128, :],
                                      in_=RH2[64:128, :])

                groups = [spT[0], spT[1], densTt]
                gwidth = [128, 128, 128]
                stats = sp.tile([128, 6], F32, tag="stats")
                nc.vector.memset(stats[:], 0.0)
                scratch = sp.tile([128, BC], F32, tag="scr")
                for gi_, (g, wd) in enumerate(zip(groups, gwidth)):
                    nc.vector.tensor_reduce(out=stats[0:wd, gi_:gi_ + 1],
                                            in_=g[0:wd, :], axis=AX.X,
                                            op=OP.add)
                    nc.vector.scalar_tensor_tensor(
                        out=scratch[0:wd, :], in0=g[0:wd, :], scalar=0.0,
                        in1=g[0:wd, :], op0=OP.add, op1=OP.mult,
                        accum_out=stats[0:wd, 3 + gi_:4 + gi_])

                cc_in = dramp.tile([128, 6], F32)
                cc_out = dramp.tile([128, 6], F32)
                nc.sync.dma_start(out=cc_in[:], in_=stats[:])
                nc.gpsimd.collective_compute(
                    "AllReduce", OP.add,
                    replica_groups=[list(range(NCORES))],
                    ins=[cc_in.opt()], outs=[cc_out.opt()])
                gstats = sp.tile([128, 6], F32, tag="gstats")
                nc.sync.dma_start(out=gstats[:], in_=cc_out[:])

                bn_gt = w.tile([128, 6], F32)
                wload(bn_gt, "bng", 128, 6)
                mu = sp.tile([128, 3], F32, tag="mu")
                nc.vector.tensor_scalar_mul(mu[:], gstats[:, 0:3], 1.0 / B)
                ex2 = sp.tile([128, 3], F32, tag="ex2")
                nc.vector.tensor_scalar_mul(ex2[:], gstats[:, 3:6], 1.0 / B)
                var = sp.tile([128, 3], F32, tag="var")
                nc.vector.tensor_tensor(out=var[:], in0=mu[:], in1=mu[:],
                                        op=OP.mult)
                nc.vector.tensor_tensor(out=var[:], in0=ex2[:], in1=var[:],
                                        op=OP.subtract)
                epst = sp.tile([128, 1], F32, tag="epst")
                nc.vector.memset(epst[:], 1e-5)
                sdv = sp.tile([128, 3], F32, tag="sdv")
                nc.scalar.activation(out=sdv[:], in_=var[:], func=AF.Sqrt,
                                     bias=epst[:], scale=1.0)
                rst = sp.tile([128, 3], F32, tag="rst")
                nc.vector.reciprocal(out=rst[:], in_=sdv[:])
                scl = sp.tile([128, 3], F32, tag="scl")
                nc.vector.tensor_tensor(out=scl[:], in0=bn_gt[:, 0:3],
                                        in1=rst[:], op=OP.mult)
                shf = sp.tile([128, 3], F32, tag="shf")
                nc.vector.tensor_tensor(out=shf[:], in0=mu[:], in1=scl[:],
                                        op=OP.mult)
                nc.vector.tensor_tensor(out=shf[:], in0=bn_gt[:, 3:6],
                                        in1=shf[:], op=OP.subtract)

                for gi_, (g, wd) in enumerate(zip(groups, gwidth)):
                    nc.vector.tensor_scalar(
                        out=g[0:wd, :], in0=g[0:wd, :],
                        scalar1=scl[0:wd, gi_:gi_ + 1],
                        scalar2=shf[0:wd, gi_:gi_ + 1],
                        op0=OP.mult, op1=OP.add)

                w1t = [w.tile([128, 256], F32, tag=f"w1t{i}", name=f"w1t{i}")
                       for i in range(3)]
                for gi_, wt in enumerate(w1t):
                    wload(wt, f"w1t{gi_}", 128, 256)
                w2t = [w.tile([128, 128], F32, tag=f"w2t{i}", name=f"w2t{i}")
                       for i in range(2)]
                for gi_, wt in enumerate(w2t):
                    wload(wt, f"w2t{gi_}", 128, 128)
                owt = w.tile([128, 1], F32)
                wload(owt, "owt", 128, 1)
                dbt = w.tile([128, 3], F32)
                wload(dbt, "dbt", 128, 3)
                obt = w.tile([1, 1], F32)
                wload(obt, "obt", 1, 1)

                h1d = [sp.tile([128, BC], F32, tag=f"h1d{i}", name=f"h1d{i}")
                       for i in range(2)]
                for mh in range(2):
                    pm = mps.tile([128, BC], F32, tag="pm1")
                    for gi_, (g, wd) in enumerate(zip(groups, gwidth)):
                        nc.tensor.matmul(
                            out=pm[:],
                            lhsT=w1t[gi_][0:wd, mh * 128:(mh + 1) * 128],
                            rhs=g[0:wd, :], start=(gi_ == 0), stop=(gi_ == 2))
                    nc.scalar.activation(out=h1d[mh][:], in_=pm[:],
                                         func=AF.Relu,
                                         bias=dbt[:, mh:mh + 1], scale=1.0)
                pm2 = mps.tile([128, BC], F32, tag="pm2")
                for mh in range(2):
                    nc.tensor.matmul(out=pm2[:], lhsT=w2t[mh][:],
                                     rhs=h1d[mh][:], start=(mh == 0),
                                     stop=(mh == 1))
                h2d = sp.tile([128, BC], F32, tag="h2d")
                nc.scalar.activation(out=h2d[:], in_=pm2[:], func=AF.Relu,
                                     bias=dbt[:, 2:3], scale=1.0)
                pmo = mps.tile([1, BC], F32, tag="pmo")
                nc.tensor.matmul(out=pmo[:], lhsT=owt[:], rhs=h2d[:],
                                 start=True, stop=True)
                res = sp.tile([1, BC], F32, tag="res")
                nc.vector.tensor_scalar(
                    out=res[:], in0=pmo[:], scalar1=obt[0:1, 0:1],
                    scalar2=None, op0=OP.add)
                nc.sync.dma_start(out=out[:], in_=res[:])

    nc.compile()
    nc._json_cache = bacc.Bacc.to_json_bytes(nc)
    return nc


# --------------------------------------------------------------------------
def _host_prep(inputs, sch):
    lens = np.asarray(inputs["hist_valid_lens"]).astype(np.int64)
    order = sch["order"]
    tmax, nts, xcols = sch["tmax"], sch["nts"], sch["xcols"]
    buf_cols = sch["buf_cols"]
    NXCH = buf_cols // 128

    embh = np.ascontiguousarray(
        np.asarray(inputs["emb"]).astype(np.float16))     # [VOCAB, 32]
    hist_item = np.asarray(inputs["hist_item"]).astype(np.int32)
    tgt = np.asarray(inputs["target_item"]).astype(np.int32)
    spf = np.asarray(inputs["sparse_feature"]).astype(np.int32)
    dense = np.asarray(inputs["dense_feature"], np.float32)

    gw = {k: np.asarray(inputs[k], np.float32) for k in
          ("gru_wih", "gru_whh", "gru_bih", "gru_bhh",
           "augru_wih", "augru_whh", "augru_bih", "augru_bhh",
           "att_w1", "att_b1", "att_w2", "att_b2", "att_w3", "att_b3",
           "bn_gamma", "bn_beta", "dnn_w1", "dnn_b1", "dnn_w2", "dnn_b2",
           "out_w", "out_b")}

    def stack_rz(wih, whh):
        m = np.zeros((128, 128), np.float32)
        m[0:64, 0:64] = wih[64:128].T      # z, x-side
        m[64:128, 0:64] = whh[64:128].T    # z, h-side
        m[0:64, 64:128] = wih[0:64].T      # r, x-side
        m[64:128, 64:128] = whh[0:64].T    # r, h-side
        return m

    def block_n(wih, whh):
        m = np.zeros((128, 128), np.float32)
        m[0:64, 0:64] = wih[128:192].T     # i_n (-> M 0:64)
        m[64:128, 64:128] = whh[128:192].T  # h_n (-> M 64:128)
        return m

    def vecs(bih, bhh):
        brz = np.zeros(128, np.float32)
        brz[0:64] = bih[64:128] + bhh[64:128]   # z
        brz[64:128] = bih[0:64] + bhh[0:64]     # r
        bhhn = np.zeros(128, np.float32)
        bhhn[64:128] = bhh[128:192]
        bihn = np.zeros(128, np.float32)
        bihn[64:128] = bih[128:192]
        return brz, bhhn, bihn

    gvecs = np.zeros((128, 6), np.float32)
    gvecs[:, 0], gvecs[:, 1], gvecs[:, 2] = vecs(gw["gru_bih"], gw["gru_bhh"])
    gvecs[:, 3], gvecs[:, 4], gvecs[:, 5] = vecs(gw["augru_bih"],
                                                 gw["augru_bhh"])

    w1 = gw["att_w1"]
    w_att = np.zeros((128, 3 * D), np.float32)
    w_att[64:128, 0:64] = w1[64:128] - w1[128:192]   # k-term
    w_att[64:128, 64:128] = w1[192:256]              # q*k-term
    w_att[64:128, 128:192] = w1[0:64] + w1[128:192]  # q-term
    attb = np.zeros((64, 2), np.float32)
    attb[:, 0] = gw["att_b1"]
    attb[0:16, 1] = gw["att_b2"]

    bn_g = np.zeros((128, 6), np.float32)
    bn_g[:, 0:3] = 1.0
    for g in range(2):
        bn_g[:, g] = gw["bn_gamma"][g * 128:(g + 1) * 128]
        bn_g[:, 3 + g] = gw["bn_beta"][g * 128:(g + 1) * 128]
    bn_g[0:DL, 2] = gw["bn_gamma"][256:272]
    bn_g[0:DL, 5] = gw["bn_beta"][256:272]
    bn_g[64:128, 2] = gw["bn_gamma"][272:336]
    bn_g[64:128, 5] = gw["bn_beta"][272:336]
    dnn_w1p = np.zeros((384, 256), np.float32)
    dnn_w1p[0:256] = gw["dnn_w1"][0:256]
    dnn_w1p[256:272] = gw["dnn_w1"][256:272]
    dnn_w1p[320:384] = gw["dnn_w1"][272:336]
    dnn_b = np.zeros((128, 3), np.float32)
    dnn_b[:, 0] = gw["dnn_b1"][0:128]
    dnn_b[:, 1] = gw["dnn_b1"][128:256]
    dnn_b[:, 2] = gw["dnn_b2"]

    wvals = dict(
        wrzg=stack_rz(gw["gru_wih"], gw["gru_whh"]),
        wng=block_n(gw["gru_wih"], gw["gru_whh"]),
        wrza=stack_rz(gw["augru_wih"], gw["augru_whh"]),
        wna=block_n(gw["augru_wih"], gw["augru_whh"]),
        gv=gvecs, watt=w_att, w2a=gw["att_w2"], w3a=gw["att_w3"],
        ab=attb, bng=bn_g,
        w1t0=dnn_w1p[0:128], w1t1=dnn_w1p[128:256], w1t2=dnn_w1p[256:384],
        w2t0=gw["dnn_w2"][0:128], w2t1=gw["dnn_w2"][128:256],
        owt=gw["out_w"], dbt=dnn_b,
        obt=gw["out_b"].reshape(1, 1))
    wflat = np.zeros(WTOT, np.float32)
    for nm, p, c in WSEGS:
        arr = np.ascontiguousarray(wvals[nm], np.float32).reshape(p, c)
        wflat[WOFF[nm]:WOFF[nm] + p * c] = arr.reshape(-1)
    wshards = wflat.view(np.float16).reshape(NCORES, 1, 2 * WSH)
    eshards = embh.reshape(NCORES, 1, ESH)

    # column -> (t, r) map for the packed x grid
    dcol_t = np.zeros(buf_cols, np.int64)
    dcol_r = np.zeros(buf_cols, np.int64)
    dcol_valid = np.zeros(buf_cols, bool)
    for t in range(tmax):
        c0, n = int(xcols[t]), int(nts[t])
        dcol_t[c0:c0 + n] = t
        dcol_r[c0:c0 + n] = np.arange(n)
        dcol_valid[c0:c0 + n] = True
    dval = np.nonzero(dcol_valid)[0]
    tt_ = dcol_t[dval]
    rr_ = dcol_r[dval]

    QOFF = 2 * NXCH
    SOFF = QOFF + 4
    NIDX = SOFF + 16

    in_maps = []
    for c in range(NCORES):
        rows = order[c::NCORES]
        idxf = np.zeros((2, buf_cols), np.int32)
        idxf[:, dval] = hist_item[rows[rr_], tt_, :].T
        xpart = idxf.reshape(2, NXCH, 128).transpose(2, 1, 0).reshape(
            128, 2 * NXCH)
        qpart = tgt[rows].reshape(2, 128, 2).transpose(1, 0, 2).reshape(
            128, 4)
        spart = spf[rows].reshape(2, 128, 8).transpose(1, 0, 2).reshape(
            128, 16)
        xidx = np.ascontiguousarray(
            np.concatenate([xpart, qpart, spart], axis=1))
        assert xidx.shape == (128, NIDX)

        densT = np.ascontiguousarray(dense[rows, :].T)
        lensv = np.ascontiguousarray(
            lens[rows].reshape(2, 128).T.astype(np.float32))

        in_maps.append(dict(
            eshard=eshards[c], wshard=wshards[c], xidx=xidx,
            densT=densT, lensv=lensv))
    return in_maps, order


_CACHE = {}


def kernel(**inputs):
    lens = np.asarray(inputs["hist_valid_lens"]).astype(np.int64)
    key = hashlib.sha1(lens.tobytes()).hexdigest()
    sch = _make_schedule(lens)
    if key not in _CACHE:
        _CACHE[key] = _build(sch)
    nc = _CACHE[key]
    in_maps, order = _host_prep(inputs, sch)
    import os, time
    trace = bool(os.environ.get("KTRACE"))
    t0 = time.perf_counter()
    res = None
    for attempt in range(3):
        try:
            res = run_bass_kernel_spmd(nc, in_maps,
                                       core_ids=list(range(NCORES)),
                                       trace=trace)
            break
        except Exception:
            if attempt == 2:
                raise
            time.sleep(2.0)
            t0 = time.perf_counter()
    kernel.last_spmd_s = time.perf_counter() - t0
    if trace and res.exec_time_ns is not None:
        print(f"HW exec time: {res.exec_time_ns} ns")
    kernel.last_res = res
    kernel.last_sch = sch
    kernel.last_maps = in_maps
    out = np.zeros((B, 1), np.float32)
    for c in range(NCORES):
        rows = order[c::NCORES]
        out[rows, 0] = res.results[c]["out"][0]
    return out


# revision 11
# speedup vs baseline: 1.2239x; 1.2239x over previous
"""DIEN forward on 8 Trainium2 NeuronCores (Bass/Tile).

Data-parallel with ragged packing (v2: device-side embedding gather).
 - Host sorts batch rows by descending hist_valid_lens, deals them to the 8
   cores round-robin over the sorted order, and builds a shared per-timestep
   active-column schedule N_t (max over cores, padded to x4).  All per-step
   shapes are compile-time constants.
 - Upload per core is only ~1.2MB: an fp16 shard of the embedding table
   (AllGathered on device into DRAM), a packed f32 weight-blob shard
   (AllGathered likewise), int32 gather indices, dense features and lens.
 - On device: indirect-DMA gathers pull embedding rows (row-major, 128 per
   instruction), PE transposes flip them feature-major into the packed x
   grid / query / sparse tiles; attention masks are built from lens + iota.
 - Scans/attention/DNN head identical to v1: feature-major packed GRU and
   AUGRU scans over ragged columns, attention scattered into batch-major
   PSUM, BatchNorm stats via AllReduce.

kernel(**inputs) takes FULL unsharded inputs, returns [B, 1] float32.
"""

import hashlib
import numpy as np

import jax
# Persistent XLA compilation cache: run_bass_kernel_spmd builds a fresh jit
# closure per call, so the in-memory executable cache always misses and the
# BIR->NEFF compile would otherwise rerun on every invocation.
jax.config.update("jax_compilation_cache_dir", "/tmp/jax_bass_cache")
jax.config.update("jax_persistent_cache_min_entry_size_bytes", -1)
jax.config.update("jax_persistent_cache_min_compile_time_secs", 0)

import concourse.bass as bass
import concourse.bacc as bacc
import concourse.tile as tile
from concourse import mybir
from concourse.bass_utils import run_bass_kernel_spmd
from concourse.masks import make_identity

F32 = mybir.dt.float32
F16 = mybir.dt.float16
I32 = mybir.dt.int32
AF = mybir.ActivationFunctionType
OP = mybir.AluOpType
AX = mybir.AxisListType

B, T, E, NF, SL, DL, VOCAB = 2048, 200, 32, 2, 8, 16, 100000
D = E * NF          # 64
NCORES = 8
BC = B // NCORES    # 256
ESH = (VOCAB // NCORES) * E   # fp16 emb-table shard elements per core


# --------------------------------------------------------------------------
def _make_schedule(lens):
    order = np.argsort(-lens, kind="stable")
    core_lens = lens[order].reshape(-1, NCORES)       # [BC, 8]
    tmax = int(lens.max())
    nts = []
    for t in range(tmax):
        n = int((core_lens > t).sum(axis=0).max())
        n = min(BC, ((n + 3) // 4) * 4)
        nts.append(max(n, 4))
    nts = np.array(nts, np.int32)
    offs = np.zeros(tmax + 1, np.int64)
    offs[1:] = np.cumsum(nts)
    m_total = int(offs[tmax])
    n0 = int(nts[0])
    xcols = np.zeros(tmax, np.int64)
    xcols[1:] = n0 + offs[:tmax - 1]                  # x(t) at h(t-1) cols
    hcols = n0 + offs[:tmax]
    xspan = int(n0 + offs[tmax - 1]) if tmax > 1 else int(nts[0])
    xspan = max(xspan, int(xcols[tmax - 1] + nts[tmax - 1]))
    buf_cols = max(n0 + m_total, ((xspan + 127) // 128) * 128 + 128)
    buf_cols = ((buf_cols + 127) // 128) * 128
    return dict(order=order, tmax=tmax, nts=nts, offs=offs, m_total=m_total,
                n0=n0, xcols=xcols, hcols=hcols, xspan=xspan,
                buf_cols=buf_cols)


def _att_chunks(sch):
    chunks, cur, w = [], [], 0
    for t in range(sch["tmax"]):
        n = int(sch["nts"][t])
        if w + n > 512 and cur:
            chunks.append(cur)
            cur, w = [], 0
        cur.append(t)
        w += n
    if cur:
        chunks.append(cur)
    return chunks


# ------------------------- weight blob layout ------------------------------
WSEGS = [  # (name, partitions, cols)
    ("wrzg", 128, 128), ("wng", 128, 128), ("wrza", 128, 128),
    ("wna", 128, 128), ("gv", 128, 6), ("watt", 128, 3 * D),
    ("w2a", D, 16), ("w3a", 16, 1), ("ab", 64, 2), ("bng", 128, 6),
    ("w1t0", 128, 256), ("w1t1", 128, 256), ("w1t2", 128, 256),
    ("w2t0", 128, 128), ("w2t1", 128, 128),
    ("owt", 128, 1), ("dbt", 128, 3), ("obt", 1, 1),
]
WOFF = {}
_off = 0
for _n, _p, _c in WSEGS:
    WOFF[_n] = _off
    _off += _p * _c
WTOT = ((_off + NCORES - 1) // NCORES) * NCORES
WSH = WTOT // NCORES


# --------------------------------------------------------------------------
class _CachedBacc(bacc.Bacc):
    _json_cache = None

    def to_json_bytes(self):
        if self._json_cache is not None:
            return self._json_cache
        return super().to_json_bytes()


def _build(sch):
    nc = _CachedBacc("TRN2", target_bir_lowering=False, debug=False,
                     num_devices=NCORES)
    tmax, nts = sch["tmax"], sch["nts"]
    hcols, xcols = sch["hcols"], sch["xcols"]
    n0, buf_cols = sch["n0"], sch["buf_cols"]
    NXCH = buf_cols // 128          # x gather chunks (128 cols each)
    QOFF = 2 * NXCH                 # idx col offsets: q then sparse
    SOFF = QOFF + 4
    NIDX = SOFF + 16
    chunks = _att_chunks(sch)

    eshard = nc.dram_tensor("eshard", [1, ESH], F16, kind="ExternalInput")
    wshard = nc.dram_tensor("wshard", [1, 2 * WSH], F16, kind="ExternalInput")
    xidx = nc.dram_tensor("xidx", [128, NIDX], I32, kind="ExternalInput")
    densT = nc.dram_tensor("densT", [DL, BC], F32, kind="ExternalInput")
    lensv = nc.dram_tensor("lensv", [128, 2], F32, kind="ExternalInput")
    out = nc.dram_tensor("out", [1, BC], F32, kind="ExternalOutput")

    with tile.TileContext(nc) as tc:
        with tc.tile_pool(name="big", bufs=1) as big, \
             tc.tile_pool(name="w", bufs=1) as w, \
             tc.tile_pool(name="s", bufs=2) as sp, \
             tc.tile_pool(name="s2", bufs=2) as sp2, \
             tc.tile_pool(name="dram", bufs=1, space="DRAM") as dramp:

            BUF = big.tile([128, buf_cols], F32)
            RH2 = big.tile([128, BC], F32)
            qT = big.tile([128, BC], F32)
            spT = [big.tile([128, BC], F32, tag=f"spT{i}", name=f"spT{i}")
                   for i in range(2)]
            attB = [big.tile([128, 256], F32, tag=f"attB{i}", name=f"attB{i}")
                    for i in range(2)]
            attT = [big.tile([128, 256], F32, tag=f"attT{i}", name=f"attT{i}")
                    for i in range(2)]

            # ------------- collectives: gather table + weights ------------
            # (collectives cannot read IO tensors; stage via Internal DRAM)
            eint = dramp.tile([1, ESH], F16)
            nc.sync.dma_start(out=eint[:], in_=eshard[:])
            egt = dramp.tile([1, NCORES * ESH], F16,
                             addr_space="Shared")
            nc.gpsimd.collective_compute(
                "AllGather", OP.bypass,
                replica_groups=[list(range(NCORES))],
                ins=[eint[:].opt()], outs=[egt.opt()])
            etab = egt[0:1, :].rearrange("o (r e) -> (o r) e", e=E)

            # weight blob rides as raw bits in an f16 AllGather (an f32
            # AllGather next to the f16 one + indirect gathers wedges NRT)
            wint = dramp.tile([1, 2 * WSH], F16)
            nc.sync.dma_start(out=wint[:], in_=wshard[:])
            wgt = dramp.tile([1, NCORES * 2 * WSH], F16,
                             addr_space="Shared")
            nc.gpsimd.collective_compute(
                "AllGather", OP.bypass,
                replica_groups=[list(range(NCORES))],
                ins=[wint[:].opt()], outs=[wgt.opt()])
            wgt32 = wgt[0:1, :].bitcast(F32)

            def wload(dst, name, p, c):
                nc.sync.dma_start(
                    out=dst[0:p, 0:c],
                    in_=wgt32[0:1, WOFF[name]:WOFF[name] + p * c].rearrange(
                        "o (p c) -> (o p) c", p=p))

            ident = w.tile([128, 128], F32)
            make_identity(nc, ident[:])
            ident16 = w.tile([128, 128], F16)
            make_identity(nc, ident16[:])
            ones1 = w.tile([1, 64], F32)
            nc.vector.memset(ones1[:], 1.0)

            wrzg = w.tile([128, 128], F32)
            wng = w.tile([128, 128], F32)
            wrza = w.tile([128, 128], F32)
            wna = w.tile([128, 128], F32)
            gv = w.tile([128, 6], F32)
            watt = w.tile([128, 3 * D], F32)
            w2a = w.tile([D, 16], F32)
            w3a = w.tile([16, 1], F32)
            ab = w.tile([64, 2], F32)
            for nm, dst in (("wrzg", wrzg), ("wng", wng), ("wrza", wrza),
                            ("wna", wna), ("gv", gv), ("watt", watt),
                            ("w2a", w2a), ("w3a", w3a), ("ab", ab)):
                p, c = dict((s[0], (s[1], s[2])) for s in WSEGS)[nm]
                wload(dst, nm, p, c)

            # ------------- indices, lens, masks ---------------------------
            XI = w.tile([128, NIDX], I32)
            nc.sync.dma_start(out=XI[:], in_=xidx[:])
            LV = w.tile([128, 2], F32)
            nc.sync.dma_start(out=LV[:], in_=lensv[:])
            iotaF = w.tile([128, T], F32)
            nc.gpsimd.iota(iotaF[:], pattern=[[1, T]], base=0,
                           channel_multiplier=0,
                           allow_small_or_imprecise_dtypes=True)
            mskT = [w.tile([128, T], F32, tag=f"mskT{i}", name=f"mskT{i}")
                    for i in range(2)]
            for i in range(2):
                nc.vector.tensor_scalar(out=mskT[i][:], in0=iotaF[:],
                                        scalar1=LV[:, i:i + 1], scalar2=None,
                                        op0=OP.is_lt)
                nc.vector.tensor_scalar(out=mskT[i][:], in0=mskT[i][:],
                                        scalar1=1.0, scalar2=1e9,
                                        op0=OP.subtract, op1=OP.mult)

            nc.vector.memset(BUF[64:128, 0:n0], 0.0)

            # ------------- device-side embedding gathers ------------------
            def gather_pair(gp, gpsum, c0, c1):
                G = gp.tile([128, 64], F16, tag="G")
                nc.gpsimd.indirect_dma_start(
                    out=G[:, 0:E], out_offset=None, in_=etab,
                    in_offset=bass.IndirectOffsetOnAxis(
                        ap=XI[:, c0:c0 + 1], axis=0))
                nc.gpsimd.indirect_dma_start(
                    out=G[:, E:2 * E], out_offset=None, in_=etab,
                    in_offset=bass.IndirectOffsetOnAxis(
                        ap=XI[:, c1:c1 + 1], axis=0))
                pt = gpsum.tile([64, 128], F16, tag="pt")
                nc.tensor.transpose(out=pt[:], in_=G[:], identity=ident16[:])
                return pt

            with tc.tile_pool(name="g", bufs=4) as gp, \
                 tc.tile_pool(name="gps", bufs=4, space="PSUM") as gpsum:
                for c in range(NXCH):
                    pt = gather_pair(gp, gpsum, 2 * c, 2 * c + 1)
                    nc.vector.tensor_copy(
                        out=BUF[0:64, c * 128:(c + 1) * 128], in_=pt[:])
                for a in range(2):
                    pt = gather_pair(gp, gpsum, QOFF + 2 * a, QOFF + 2 * a + 1)
                    nc.vector.tensor_copy(
                        out=qT[64:128, a * 128:(a + 1) * 128], in_=pt[:])
                for a in range(2):
                    for j in range(4):
                        c0 = SOFF + a * 8 + 2 * j
                        pt = gather_pair(gp, gpsum, c0, c0 + 1)
                        nc.vector.tensor_copy(
                            out=spT[j // 2][(j % 2) * 64:(j % 2) * 64 + 64,
                                            a * 128:(a + 1) * 128],
                            in_=pt[:])

            # ---------------- scan step ---------------------------------
            def scan_step(pool, t, rhs_buf, rhs_col, wrz, wn, vo, out_buf,
                          out_col, att_rhs=None):
                n = int(nts[t])
                pA = pool.tile([128, 256], F32, tag="pA")
                pB = pool.tile([128, 256], F32, tag="pB")
                rhs = rhs_buf[:, rhs_col:rhs_col + n]
                nc.tensor.matmul(out=pA[:, 0:n], lhsT=wrz[:], rhs=rhs,
                                 start=True, stop=True)
                nc.tensor.matmul(out=pB[:, 0:n], lhsT=wn[:], rhs=rhs,
                                 start=True, stop=True)
                srz = sp.tile([128, 256], F32, tag="srz")
                nc.scalar.activation(out=srz[:, 0:n], in_=pA[:, 0:n],
                                     func=AF.Sigmoid,
                                     bias=gv[:, vo:vo + 1], scale=1.0)
                t1 = sp.tile([128, 256], F32, tag="t1")
                nc.vector.scalar_tensor_tensor(
                    out=t1[64:128, 0:n], in0=pB[64:128, 0:n],
                    scalar=gv[64:128, vo + 1:vo + 2],
                    in1=srz[64:128, 0:n], op0=OP.add, op1=OP.mult)
                t2 = sp.tile([128, 256], F32, tag="t2")
                nc.vector.tensor_tensor(out=t2[64:128, 0:n],
                                        in0=t1[64:128, 0:n],
                                        in1=pB[0:64, 0:n], op=OP.add)
                nt = sp.tile([128, 256], F32, tag="nt")
                nc.scalar.activation(out=nt[64:128, 0:n], in_=t2[64:128, 0:n],
                                     func=AF.Tanh,
                                     bias=gv[64:128, vo + 2:vo + 3], scale=1.0)
                pD = pool.tile([64, 256], F32, tag="pD")
                h_prev = rhs_buf[64:128, rhs_col:rhs_col + n]
                et = sp2.tile([128, 256], F32, tag="et")
                if att_rhs is None:
                    # GRU: h' = n + z*(h - n)
                    nc.vector.tensor_tensor(out=pD[0:64, 0:n], in0=h_prev,
                                            in1=nt[64:128, 0:n],
                                            op=OP.subtract)
                    nc.vector.tensor_tensor(out=et[64:128, 0:n],
                                            in0=pD[0:64, 0:n],
                                            in1=srz[0:64, 0:n], op=OP.mult)
                    nc.vector.tensor_tensor(
                        out=out_buf[64:128, out_col:out_col + n],
                        in0=et[64:128, 0:n], in1=nt[64:128, 0:n], op=OP.add)
                else:
                    # AUGRU: h' = h + att*z*(n - h)
                    nc.vector.tensor_tensor(out=pD[0:64, 0:n],
                                            in0=nt[64:128, 0:n],
                                            in1=h_prev, op=OP.subtract)
                    pAtt = pool.tile([64, 256], F32, tag="pAtt")
                    nc.tensor.matmul(out=pAtt[:, 0:n], lhsT=ones1[:],
                                     rhs=att_rhs, start=True, stop=True)
                    zt = sp2.tile([128, 256], F32, tag="zt")
                    nc.vector.tensor_tensor(out=zt[0:64, 0:n],
                                            in0=pAtt[0:64, 0:n],
                                            in1=srz[0:64, 0:n], op=OP.mult)
                    nc.vector.tensor_tensor(out=et[64:128, 0:n],
                                            in0=pD[0:64, 0:n],
                                            in1=zt[0:64, 0:n], op=OP.mult)
                    nc.vector.tensor_tensor(
                        out=out_buf[64:128, out_col:out_col + n],
                        in0=et[64:128, 0:n],
                        in1=rhs_buf[64:128, rhs_col:rhs_col + n], op=OP.add)

            # ---------------- GRU scan ----------------------------------
            with tc.tile_pool(name="sps", bufs=2, space="PSUM") as sps:
                for t in range(tmax):
                    scan_step(sps, t, BUF, int(xcols[t]), wrzg, wng, 0,
                              BUF, int(hcols[t]))

            # ---------------- attention ---------------------------------
            with tc.tile_pool(name="apsB", bufs=1, space="PSUM") as apsB, \
                 tc.tile_pool(name="aps", bufs=2, space="PSUM") as aps:
                psB = [apsB.tile([128, T], F32, tag=f"psB{i}", name=f"psB{i}")
                       for i in range(2)]
                nc.vector.memset(psB[0][:], 0.0)
                nc.vector.memset(psB[1][:], 0.0)

                for ch in chunks:
                    wch = int(sum(int(nts[t]) for t in ch))
                    qk = sp.tile([128, 512], F32, tag="qk")
                    col = 0
                    for t in ch:
                        n = int(nts[t])
                        hc = int(hcols[t])
                        nc.vector.tensor_tensor(
                            out=qk[64:128, col:col + n],
                            in0=BUF[64:128, hc:hc + n],
                            in1=qT[64:128, 0:n], op=OP.mult)
                        col += n
                    pL1 = aps.tile([64, 512], F32, tag="pL1")
                    col = 0
                    for t in ch:
                        n = int(nts[t])
                        hc = int(hcols[t])
                        nc.tensor.matmul(out=pL1[:, col:col + n],
                                         lhsT=watt[64:128, 0:64],
                                         rhs=BUF[64:128, hc:hc + n],
                                         start=True, stop=False)
                        nc.tensor.matmul(out=pL1[:, col:col + n],
                                         lhsT=watt[64:128, 64:128],
                                         rhs=qk[64:128, col:col + n],
                                         start=False, stop=False)
                        nc.tensor.matmul(out=pL1[:, col:col + n],
                                         lhsT=watt[64:128, 128:192],
                                         rhs=qT[64:128, 0:n],
                                         start=False, stop=True)
                        col += n
                    h1 = sp.tile([64, 512], F32, tag="h1")
                    nc.scalar.activation(out=h1[:, 0:wch], in_=pL1[:, 0:wch],
                                         func=AF.Relu, bias=ab[:, 0:1],
                                         scale=1.0)
                    pL2 = aps.tile([16, 512], F32, tag="pL2")
                    nc.tensor.matmul(out=pL2[:, 0:wch], lhsT=w2a[:],
                                     rhs=h1[:, 0:wch], start=True, stop=True)
                    h2 = sp.tile([16, 512], F32, tag="h2")
                    nc.scalar.activation(out=h2[:, 0:wch], in_=pL2[:, 0:wch],
                                         func=AF.Relu, bias=ab[0:16, 1:2],
                                         scale=1.0)
                    col = 0
                    for t in ch:
                        n = int(nts[t])
                        for piece in range(2):
                            lo = piece * 128
                            if lo >= n:
                                break
                            pw = min(128, n - lo)
                            nc.tensor.matmul(
                                out=psB[piece][0:pw, t:t + 1],
                                lhsT=h2[:, col + lo:col + lo + pw],
                                rhs=w3a[:], start=True, stop=True)
                        col += n

                # softmax (batch-major)
                for i in range(2):
                    sc_t = sp.tile([128, T], F32, tag="sct")
                    nc.vector.tensor_tensor(out=sc_t[:], in0=psB[i][:],
                                            in1=mskT[i][:], op=OP.add)
                    mx = sp.tile([128, 1], F32, tag="mx")
                    nc.vector.tensor_reduce(out=mx[:], in_=sc_t[:],
                                            axis=AX.X, op=OP.max)
                    nmx = sp.tile([128, 1], F32, tag="nmx")
                    nc.vector.tensor_scalar_mul(nmx[:], mx[:], -1.0)
                    ex = sp.tile([128, 256], F32, tag="ex")
                    nc.vector.memset(ex[:], 0.0)
                    nc.scalar.activation(out=ex[:, 0:T], in_=sc_t[:],
                                         func=AF.Exp, bias=nmx[:], scale=1.0)
                    sm = sp.tile([128, 1], F32, tag="sm")
                    nc.vector.tensor_reduce(out=sm[:], in_=ex[:, 0:T],
                                            axis=AX.X, op=OP.add)
                    rs = sp.tile([128, 1], F32, tag="rs")
                    nc.vector.reciprocal(out=rs[:], in_=sm[:])
                    nc.vector.memset(attB[i][:], 0.0)
                    nc.vector.tensor_scalar(
                        out=attB[i][:, 0:T], in0=ex[:, 0:T], scalar1=rs[:],
                        scalar2=None, op0=OP.mult)

                # transpose attB -> attT (rows = t, cols = r)
                for th in range(2):
                    tw = 128 if th == 0 else T - 128
                    for rh in range(2):
                        pat = aps.tile([128, 128], F32, tag="pAT")
                        nc.tensor.transpose(
                            out=pat[0:tw, :],
                            in_=attB[rh][:, th * 128:th * 128 + tw],
                            identity=ident[:])
                        nc.vector.tensor_copy(
                            out=attT[th][0:tw, rh * 128:(rh + 1) * 128],
                            in_=pat[0:tw, :])

            # ---------------- AUGRU scan --------------------------------
            nc.vector.memset(RH2[:], 0.0)
            with tc.tile_pool(name="aups", bufs=2, space="PSUM") as aups, \
                 tc.tile_pool(name="strp", bufs=2) as strp:
                nstrip = (tmax + 7) // 8
                for s in range(nstrip):
                    t0 = s * 8
                    t1s = min(t0 + 8, tmax)
                    rows = t1s - t0
                    strip = strp.tile([1, 8 * 256], F32, tag="strip")
                    th = t0 // 128
                    r0 = t0 - th * 128
                    nc.sync.dma_start(
                        out=strip[0:1, 0:rows * 256].rearrange(
                            "o (t r) -> o t r", t=rows),
                        in_=attT[th][r0:r0 + rows, :])
                    for t in range(t0, t1s):
                        n = int(nts[t])
                        hc = int(hcols[t])
                        nc.gpsimd.tensor_copy(out=RH2[0:64, 0:n],
                                              in_=BUF[64:128, hc:hc + n])
                        arhs = strip[0:1, (t - t0) * 256:(t - t0) * 256 + n]
                        scan_step(aups, t, RH2, 0, wrza, wna, 3, RH2, 0,
                                  att_rhs=arhs)

            # ---------------- DNN head ----------------------------------
            with tc.tile_pool(name="mps", bufs=2, space="PSUM") as mps:
                densTt = big.tile([128, BC], F32, tag="densTt")
                nc.vector.memset(densTt[:], 0.0)
                nc.sync.dma_start(out=densTt[0:DL, :], in_=densT[:])
                nc.vector.tensor_copy(out=densTt[64:128, :],
                                      in_=RH2[64:128, :])

                groups = [spT[0], spT[1], densTt]
                gwidth = [128, 128, 128]
                stats = sp.tile([128, 6], F32, tag="stats")
                nc.vector.memset(stats[:], 0.0)
                scratch = sp.tile([128, BC], F32, tag="scr")
                for gi_, (g, wd) in enumerate(zip(groups, gwidth)):
                    nc.vector.tensor_reduce(out=stats[0:wd, gi_:gi_ + 1],
                                            in_=g[0:wd, :], axis=AX.X,
                                            op=OP.add)
                    nc.vector.scalar_tensor_tensor(
                        out=scratch[0:wd, :], in0=g[0:wd, :], scalar=0.0,
                        in1=g[0:wd, :], op0=OP.add, op1=OP.mult,
                        accum_out=stats[0:wd, 3 + gi_:4 + gi_])

                cc_in = dramp.tile([128, 6], F32)
                cc_out = dramp.tile([128, 6], F32)
                nc.sync.dma_start(out=cc_in[:], in_=stats[:])
                nc.gpsimd.collective_compute(
                    "AllReduce", OP.add,
                    replica_groups=[list(range(NCORES))],
                    ins=[cc_in.opt()], outs=[cc_out.opt()])
                gstats = sp.tile([128, 6], F32, tag="gstats")
                nc.sync.dma_start(out=gstats[:], in_=cc_out[:])

                bn_gt = w.tile([128, 6], F32)
                wload(bn_gt, "bng", 128, 6)
                mu = sp.tile([128, 3], F32, tag="mu")
                nc.vector.tensor_scalar_mul(mu[:], gstats[:, 0:3], 1.0 / B)
                ex2 = sp.tile([128, 3], F32, tag="ex2")
                nc.vector.tensor_scalar_mul(ex2[:], gstats[:, 3:6], 1.0 / B)
                var = sp.tile([128, 3], F32, tag="var")
                nc.vector.tensor_tensor(out=var[:], in0=mu[:], in1=mu[:],
                                        op=OP.mult)
                nc.vector.tensor_tensor(out=var[:], in0=ex2[:], in1=var[:],
                                        op=OP.subtract)
                epst = sp.tile([128, 1], F32, tag="epst")
                nc.vector.memset(epst[:], 1e-5)
                sdv = sp.tile([128, 3], F32, tag="sdv")
                nc.scalar.activation(out=sdv[:], in_=var[:], func=AF.Sqrt,
                                     bias=epst[:], scale=1.0)
                rst = sp.tile([128, 3], F32, tag="rst")
                nc.vector.reciprocal(out=rst[:], in_=sdv[:])
                scl = sp.tile([128, 3], F32, tag="scl")
                nc.vector.tensor_tensor(out=scl[:], in0=bn_gt[:, 0:3],
                                        in1=rst[:], op=OP.mult)
                shf = sp.tile([128, 3], F32, tag="shf")
                nc.vector.tensor_tensor(out=shf[:], in0=mu[:], in1=scl[:],
                                        op=OP.mult)
                nc.vector.tensor_tensor(out=shf[:], in0=bn_gt[:, 3:6],
                                        in1=shf[:], op=OP.subtract)

                for gi_, (g, wd) in enumerate(zip(groups, gwidth)):
                    nc.vector.tensor_scalar(
                        out=g[0:wd, :], in0=g[0:wd, :],
                        scalar1=scl[0:wd, gi_:gi_ + 1],
                        scalar2=shf[0:wd, gi_:gi_ + 1],
                        op0=OP.mult, op1=OP.add)

                w1t = [w.tile([128, 256], F32, tag=f"w1t{i}", name=f"w1t{i}")
                       for i in range(3)]
                for gi_, wt in enumerate(w1t):
                    wload(wt, f"w1t{gi_}", 128, 256)
                w2t = [w.tile([128, 128], F32, tag=f"w2t{i}", name=f"w2t{i}")
                       for i in range(2)]
                for gi_, wt in enumerate(w2t):
                    wload(wt, f"w2t{gi_}", 128, 128)
                owt = w.tile([128, 1], F32)
                wload(owt, "owt", 128, 1)
                dbt = w.tile([128, 3], F32)
                wload(dbt, "dbt", 128, 3)
                obt = w.tile([1, 1], F32)
                wload(obt, "obt", 1, 1)

                h1d = [sp.tile([128, BC], F32, tag=f"h1d{i}", name=f"h1d{i}")
                       for i in range(2)]
                for mh in range(2):
                    pm = mps.tile([128, BC], F32, tag="pm1")
                    for gi_, (g, wd) in enumerate(zip(groups, gwidth)):
                        nc.tensor.matmul(
                            out=pm[:],
                            lhsT=w1t[gi_][0:wd, mh * 128:(mh + 1) * 128],
                            rhs=g[0:wd, :], start=(gi_ == 0), stop=(gi_ == 2))
                    nc.scalar.activation(out=h1d[mh][:], in_=pm[:],
                                         func=AF.Relu,
                                         bias=dbt[:, mh:mh + 1], scale=1.0)
                pm2 = mps.tile([128, BC], F32, tag="pm2")
                for mh in range(2):
                    nc.tensor.matmul(out=pm2[:], lhsT=w2t[mh][:],
                                     rhs=h1d[mh][:], start=(mh == 0),
                                     stop=(mh == 1))
                h2d = sp.tile([128, BC], F32, tag="h2d")
                nc.scalar.activation(out=h2d[:], in_=pm2[:], func=AF.Relu,
                                     bias=dbt[:, 2:3], scale=1.0)
                pmo = mps.tile([1, BC], F32, tag="pmo")
                nc.tensor.matmul(out=pmo[:], lhsT=owt[:], rhs=h2d[:],
                                 start=True, stop=True)
                res = sp.tile([1, BC], F32, tag="res")
                nc.vector.tensor_scalar(
                    out=res[:], in0=pmo[:], scalar1=obt[0:1, 0:1],
                    scalar2=None, op0=OP.add)
                nc.sync.dma_start(out=out[:], in_=res[:])

    nc.compile()
    nc._json_cache = bacc.Bacc.to_json_bytes(nc)
    return nc


# --------------------------------------------------------------------------
def _host_prep(inputs, sch):
    lens = np.asarray(inputs["hist_valid_lens"]).astype(np.int64)
    order = sch["order"]
    tmax, nts, xcols = sch["tmax"], sch["nts"], sch["xcols"]
    buf_cols = sch["buf_cols"]
    NXCH = buf_cols // 128

    embh = np.ascontiguousarray(
        np.asarray(inputs["emb"]).astype(np.float16))     # [VOCAB, 32]
    hist_item = np.asarray(inputs["hist_item"]).astype(np.int32)
    tgt = np.asarray(inputs["target_item"]).astype(np.int32)
    spf = np.asarray(inputs["sparse_feature"]).astype(np.int32)
    dense = np.asarray(inputs["dense_feature"], np.float32)

    gw = {k: np.asarray(inputs[k], np.float32) for k in
          ("gru_wih", "gru_whh", "gru_bih", "gru_bhh",
           "augru_wih", "augru_whh", "augru_bih", "augru_bhh",
           "att_w1", "att_b1", "att_w2", "att_b2", "att_w3", "att_b3",
           "bn_gamma", "bn_beta", "dnn_w1", "dnn_b1", "dnn_w2", "dnn_b2",
           "out_w", "out_b")}

    def stack_rz(wih, whh):
        m = np.zeros((128, 128), np.float32)
        m[0:64, 0:64] = wih[64:128].T      # z, x-side
        m[64:128, 0:64] = whh[64:128].T    # z, h-side
        m[0:64, 64:128] = wih[0:64].T      # r, x-side
        m[64:128, 64:128] = whh[0:64].T    # r, h-side
        return m

    def block_n(wih, whh):
        m = np.zeros((128, 128), np.float32)
        m[0:64, 0:64] = wih[128:192].T     # i_n (-> M 0:64)
        m[64:128, 64:128] = whh[128:192].T  # h_n (-> M 64:128)
        return m

    def vecs(bih, bhh):
        brz = np.zeros(128, np.float32)
        brz[0:64] = bih[64:128] + bhh[64:128]   # z
        brz[64:128] = bih[0:64] + bhh[0:64]     # r
        bhhn = np.zeros(128, np.float32)
        bhhn[64:128] = bhh[128:192]
        bihn = np.zeros(128, np.float32)
        bihn[64:128] = bih[128:192]
        return brz, bhhn, bihn

    gvecs = np.zeros((128, 6), np.float32)
    gvecs[:, 0], gvecs[:, 1], gvecs[:, 2] = vecs(gw["gru_bih"], gw["gru_bhh"])
    gvecs[:, 3], gvecs[:, 4], gvecs[:, 5] = vecs(gw["augru_bih"],
                                                 gw["augru_bhh"])

    w1 = gw["att_w1"]
    w_att = np.zeros((128, 3 * D), np.float32)
    w_att[64:128, 0:64] = w1[64:128] - w1[128:192]   # k-term
    w_att[64:128, 64:128] = w1[192:256]              # q*k-term
    w_att[64:128, 128:192] = w1[0:64] + w1[128:192]  # q-term
    attb = np.zeros((64, 2), np.float32)
    attb[:, 0] = gw["att_b1"]
    attb[0:16, 1] = gw["att_b2"]

    bn_g = np.zeros((128, 6), np.float32)
    bn_g[:, 0:3] = 1.0
    for g in range(2):
        bn_g[:, g] = gw["bn_gamma"][g * 128:(g + 1) * 128]
        bn_g[:, 3 + g] = gw["bn_beta"][g * 128:(g + 1) * 128]
    bn_g[0:DL, 2] = gw["bn_gamma"][256:272]
    bn_g[0:DL, 5] = gw["bn_beta"][256:272]
    bn_g[64:128, 2] = gw["bn_gamma"][272:336]
    bn_g[64:128, 5] = gw["bn_beta"][272:336]
    dnn_w1p = np.zeros((384, 256), np.float32)
    dnn_w1p[0:256] = gw["dnn_w1"][0:256]
    dnn_w1p[256:272] = gw["dnn_w1"][256:272]
    dnn_w1p[320:384] = gw["dnn_w1"][272:336]
    dnn_b = np.zeros((128, 3), np.float32)
    dnn_b[:, 0] = gw["dnn_b1"][0:128]
    dnn_b[:, 1] = gw["dnn_b1"][128:256]
    dnn_b[:, 2] = gw["dnn_b2"]

    wvals = dict(
        wrzg=stack_rz(gw["gru_wih"], gw["gru_whh"]),
        wng=block_n(gw["gru_wih"], gw["gru_whh"]),
        wrza=stack_rz(gw["augru_wih"], gw["augru_whh"]),
        wna=block_n(gw["augru_wih"], gw["augru_whh"]),
        gv=gvecs, watt=w_att, w2a=gw["att_w2"], w3a=gw["att_w3"],
        ab=attb, bng=bn_g,
        w1t0=dnn_w1p[0:128], w1t1=dnn_w1p[128:256], w1t2=dnn_w1p[256:384],
        w2t0=gw["dnn_w2"][0:128], w2t1=gw["dnn_w2"][128:256],
        owt=gw["out_w"], dbt=dnn_b,
        obt=gw["out_b"].reshape(1, 1))
    wflat = np.zeros(WTOT, np.float32)
    for nm, p, c in WSEGS:
        arr = np.ascontiguousarray(wvals[nm], np.float32).reshape(p, c)
        wflat[WOFF[nm]:WOFF[nm] + p * c] = arr.reshape(-1)
    wshards = wflat.view(np.float16).reshape(NCORES, 1, 2 * WSH)
    eshards = embh.reshape(NCORES, 1, ESH)

    # column -> (t, r) map for the packed x grid
    dcol_t = np.zeros(buf_cols, np.int64)
    dcol_r = np.zeros(buf_cols, np.int64)
    dcol_valid = np.zeros(buf_cols, bool)
    for t in range(tmax):
        c0, n = int(xcols[t]), int(nts[t])
        dcol_t[c0:c0 + n] = t
        dcol_r[c0:c0 + n] = np.arange(n)
        dcol_valid[c0:c0 + n] = True
    dval = np.nonzero(dcol_valid)[0]
    tt_ = dcol_t[dval]
    rr_ = dcol_r[dval]

    QOFF = 2 * NXCH
    SOFF = QOFF + 4
    NIDX = SOFF + 16

    in_maps = []
    for c in range(NCORES):
        rows = order[c::NCORES]
        idxf = np.zeros((2, buf_cols), np.int32)
        idxf[:, dval] = hist_item[rows[rr_], tt_, :].T
        xpart = idxf.reshape(2, NXCH, 128).transpose(2, 1, 0).reshape(
            128, 2 * NXCH)
        qpart = tgt[rows].reshape(2, 128, 2).transpose(1, 0, 2).reshape(
            128, 4)
        spart = spf[rows].reshape(2, 128, 8).transpose(1, 0, 2).reshape(
            128, 16)
        xidx = np.ascontiguousarray(
            np.concatenate([xpart, qpart, spart], axis=1))
        assert xidx.shape == (128, NIDX)

        densT = np.ascontiguousarray(dense[rows, :].T)
        lensv = np.ascontiguousarray(
            lens[rows].reshape(2, 128).T.astype(np.float32))

        in_maps.append(dict(
            eshard=eshards[c], wshard=wshards[c], xidx=xidx,
            densT=densT, lensv=lensv))
    return in_maps, order


_CACHE = {}


def kernel(**inputs):
    lens = np.asarray(inputs["hist_valid_lens"]).astype(np.int64)
    key = hashlib.sha1(lens.tobytes()).hexdigest()
    sch = _make_schedule(lens)
    if key not in _CACHE:
        _CACHE[key] = _build(sch)
    nc = _CACHE[key]
    in_maps, order = _host_prep(inputs, sch)
    import os, time
    trace = bool(os.environ.get("KTRACE"))
    t0 = time.perf_counter()
    res = None
    for attempt in range(3):
        try:
            res = run_bass_kernel_spmd(nc, in_maps,
                                       core_ids=list(range(NCORES)),
                                       trace=trace)
            break
        except Exception:
            if attempt == 2:
                raise
            time.sleep(2.0)
            t0 = time.perf_counter()
    kernel.last_spmd_s = time.perf_counter() - t0
    if trace and res.exec_time_ns is not None:
        print(f"HW exec time: {res.exec_time_ns} ns")
    kernel.last_res = res
    kernel.last_sch = sch
    kernel.last_maps = in_maps
    out = np.zeros((B, 1), np.float32)
    for c in range(NCORES):
        rows = order[c::NCORES]
        out[rows, 0] = res.results[c]["out"][0]
    return out
